# revision 1
# baseline (speedup 1.0000x reference)
"""GCN encoder (3-layer GCNConv + LN + relu, plus sparse residual) on 8 trn2 NeuronCores.

Strategy (matches the sharding hint):
  - Nodes are sharded across 8 cores by contiguous id range; edges are owned by
    their destination node's core so every scatter-add is core-local.
  - Key algebra: matmuls commute out of the aggregations,
        gcn_agg = (sum_e norm_e * h[src]) @ W      (not sum of (h@W)[src])
        residual = (sum_e val_e * x_org[dst]) @ Wres
    so the device only ever gathers RAW node-feature rows (256B each) with the
    dma_gather SWDGE ucode, scatter-adds them into 128-node blocks with
    one-hot PE matmuls accumulated in PSUM, and runs small 64x64 matmuls on
    the node-level results.
  - Per layer, each core computes its shard of g = h * dinv and an AllGather
    replicates the full g table into every core's HBM for the next gather.
  - dma_gather indices are int16, so gather sources are grouped into
    32768-row windows of the table; edges are sorted (window, dst-block) and
    chunked into 128-edge chunks (padded at window/block boundaries).

kernel() is self-contained: it derives everything from the inputs at call time.
"""

import os

import numpy as np

P = 128
D = 64
NCORES = 8
WIN = 32768          # dma_gather int16 index window (table rows)
C_BUDGET = 32        # chunks (of 128 edges) per dma_gather instruction
ST_BATCH = 8         # chunks per one-hot build DVE op
LN_EPS = 1e-5
PAD_DST = 300.0      # pad dst_local value (matches no iota column)


# ----------------------------------------------------------------------------
# Host-side preprocessing
# ----------------------------------------------------------------------------

def _edge_plan(seg_local, gat_gid, vals, TNB, nwin):
    """Sort one core's edges by (gather window, dst block)."""
    w = gat_gid // WIN
    b = seg_local // P
    order = np.lexsort((seg_local, b, w))
    return dict(
        w=w[order], b=b[order],
        idx16=(gat_gid - w * WIN)[order].astype(np.int16),
        dstf=(seg_local % P)[order].astype(np.float32),
        val=(vals[order] if vals is not None else None),
        counts=np.bincount(w * TNB + b, minlength=nwin * TNB),
    )


def _pack_side(plans, TNB, nwin, with_val):
    """Equalize chunk counts across cores; emit flat per-core data arrays in
    gather-batch layout plus the shared compile-time schedule.

    Schedule: list of (window, runs); runs = [(block, n_chunks, first, last)].
    """
    counts = np.stack([p["counts"] for p in plans])
    nch = (-(-np.max(counts, 0) // P)).reshape(nwin, TNB)

    batches = []
    for w in range(nwin):
        cur, room = [], C_BUDGET
        for b in range(TNB):
            n = int(nch[w, b])
            first = True
            while n > 0:
                take = min(n, room)
                cur.append((b, take, first, n - take == 0))
                first = False
                n -= take
                room -= take
                if room == 0:
                    batches.append((w, cur))
                    cur, room = [], C_BUDGET
        if cur:
            batches.append((w, cur))

    idx_packed, dst_packed, val_packed = [], [], []
    for p in plans:
        cnt = p["counts"].reshape(nwin, TNB)
        starts = np.zeros(nwin * TNB + 1, np.int64)
        np.cumsum(p["counts"], out=starts[1:])
        starts = starts[:-1].reshape(nwin, TNB)
        consumed = {}
        idx_parts, dst_parts, val_parts = [], [], []
        for w, runs in batches:
            bi, bd, bv = [], [], []
            for (b, take, first, last) in runs:
                done = consumed.get((w, b), 0)
                s = int(starts[w, b]) + done * P
                e = min(int(starts[w, b]) + int(cnt[w, b]), s + take * P)
                n_real = max(0, e - s)
                ii = np.zeros(take * P, np.int16)
                dd = np.full(take * P, PAD_DST, np.float32)
                ii[:n_real] = p["idx16"][s:s + n_real]
                dd[:n_real] = p["dstf"][s:s + n_real]
                bi.append(ii)
                bd.append(dd)
                if with_val:
                    vv = np.zeros(take * P, np.float32)
                    vv[:n_real] = p["val"][s:s + n_real]
                    bv.append(vv)
                consumed[(w, b)] = done + take
            flat = np.concatenate(bi)
            NI = len(flat)
            a = flat.reshape(NI // 16, 16).T          # wrap into 16 partitions
            idx_parts.append(np.ascontiguousarray(np.tile(a, (8, 1))).ravel())
            flat = np.concatenate(bd)
            dst_parts.append(np.ascontiguousarray(flat.reshape(-1, P).T).ravel())
            if with_val:
                flat = np.concatenate(bv)
                val_parts.append(np.ascontiguousarray(flat.reshape(-1, P).T).ravel())
        idx_packed.append(np.concatenate(idx_parts))
        dst_packed.append(np.concatenate(dst_parts))
        if with_val:
            val_packed.append(np.concatenate(val_parts))

    return dict(
        idx=idx_packed, dst=dst_packed,
        val=val_packed if with_val else None,
        batches=batches,
    )


def _preprocess(x, x_org, adj_values, edge_index):
    N = x.shape[0]
    assert N % NCORES == 0
    PER = N // NCORES
    TNB = -(-PER // P)
    PAD_N = TNB * P
    GROWS = NCORES * PAD_N
    src = np.asarray(edge_index[0], dtype=np.int64)
    dst = np.asarray(edge_index[1], dtype=np.int64)
    adj_values = np.asarray(adj_values, dtype=np.float32)

    deg_in = np.bincount(dst, minlength=N)
    dinv = (1.0 / np.sqrt(deg_in + 1.0)).astype(np.float32)

    # g-table row of node v: shard (v // PER), partition-major within shard
    ids = np.arange(N)
    r_of = ids % PER
    gid = (ids // PER) * PAD_N + (r_of % P) * TNB + (r_of // P)

    nwin_c = -(-GROWS // WIN)
    nwin_r = -(-N // WIN)

    conv_plans, res_plans = [], []
    for c in range(NCORES):
        m = (dst >= c * PER) & (dst < (c + 1) * PER)
        conv_plans.append(_edge_plan(dst[m] - c * PER, gid[src[m]], None, TNB, nwin_c))
        m = (src >= c * PER) & (src < (c + 1) * PER)
        res_plans.append(
            _edge_plan(src[m] - c * PER, dst[m], adj_values[m], TNB, nwin_r))

    conv = _pack_side(conv_plans, TNB, nwin_c, with_val=False)
    res = _pack_side(res_plans, TNB, nwin_r, with_val=True)

    x = np.asarray(x, np.float32)
    x_lay, dinv_lay = [], []
    for c in range(NCORES):
        xm = np.zeros((PAD_N, D), np.float32)
        dm = np.zeros(PAD_N, np.float32)
        xm[:PER] = x[c * PER:(c + 1) * PER]
        dm[:PER] = dinv[c * PER:(c + 1) * PER]
        x_lay.append(xm.reshape(TNB, P, D).transpose(1, 0, 2).reshape(P, TNB * D).copy())
        dinv_lay.append(dm.reshape(TNB, P).transpose(1, 0).copy())

    return dict(
        N=N, PER=PER, TNB=TNB, PAD_N=PAD_N, GROWS=GROWS,
        nwin_c=nwin_c, nwin_r=nwin_r, conv=conv, res=res,
        x_lay=x_lay, dinv_lay=dinv_lay,
    )


# ----------------------------------------------------------------------------
# Bass kernel builder
# ----------------------------------------------------------------------------

def _build_bass(meta):
    import concourse.bacc as bacc
    import concourse.bass as bass  # noqa: F401
    import concourse.mybir as mybir
    import concourse.tile as tile
    from concourse.masks import make_identity

    dt = mybir.dt
    Alu = mybir.AluOpType
    Act = mybir.ActivationFunctionType
    f32 = dt.float32

    N = meta["N"]
    TNB = meta["TNB"]
    GROWS = meta["GROWS"]

    nc = bacc.Bacc(
        "TRN2",
        target_bir_lowering=False,
        debug=False,
        enable_asserts=False,
        num_devices=NCORES,
    )

    # ---- I/O ----
    x_lay = nc.dram_tensor("x_lay", [P, TNB * D], f32, kind="ExternalInput")
    dinv_lay = nc.dram_tensor("dinv_lay", [P, TNB], f32, kind="ExternalInput")
    x_org = nc.dram_tensor("x_org", [N, D], f32, kind="ExternalInput")
    conv_idx = nc.dram_tensor("conv_idx", [len(meta["conv"]["idx"][0])], dt.int16,
                              kind="ExternalInput")
    conv_dst = nc.dram_tensor("conv_dst", [len(meta["conv"]["dst"][0])], f32,
                              kind="ExternalInput")
    res_idx = nc.dram_tensor("res_idx", [len(meta["res"]["idx"][0])], dt.int16,
                             kind="ExternalInput")
    res_dst = nc.dram_tensor("res_dst", [len(meta["res"]["dst"][0])], f32,
                             kind="ExternalInput")
    res_val = nc.dram_tensor("res_val", [len(meta["res"]["val"][0])], f32,
                             kind="ExternalInput")
    iota_in = nc.dram_tensor("iota_in", [P, P], f32, kind="ExternalInput")
    Wi = nc.dram_tensor("Wi", [D, D], f32, kind="ExternalInput")
    convW = nc.dram_tensor("convW", [3, D, D], f32, kind="ExternalInput")
    Wl = nc.dram_tensor("Wl", [D, D], f32, kind="ExternalInput")
    Wres = nc.dram_tensor("Wres", [D, D], f32, kind="ExternalInput")
    bi_rep = nc.dram_tensor("bi_rep", [P, D], f32, kind="ExternalInput")
    bl_rep = nc.dram_tensor("bl_rep", [P, D], f32, kind="ExternalInput")
    convb_rep = nc.dram_tensor("convb_rep", [3, P, D], f32, kind="ExternalInput")
    lng_rep = nc.dram_tensor("lng_rep", [3, P, D], f32, kind="ExternalInput")
    lnb_rep = nc.dram_tensor("lnb_rep", [3, P, D], f32, kind="ExternalInput")

    out_sh = nc.dram_tensor("out_sh", [P, TNB * D], f32, kind="ExternalOutput")
    res_sh = nc.dram_tensor("res_sh", [P, TNB * D], f32, kind="ExternalOutput")

    # ---- internal DRAM ----
    g_in = nc.dram_tensor("g_in", [P * TNB * D], f32)
    g_table = nc.dram_tensor("g_table", [GROWS * D], f32, addr_space="Shared")
    g_rows = g_table[:].rearrange("(r d) -> r d", d=D)

    with tile.TileContext(nc) as tc:
        cst = tc.alloc_tile_pool(name="cst", bufs=1)
        big = tc.alloc_tile_pool(name="big", bufs=1)
        gat = tc.alloc_tile_pool(name="gat", bufs=3)
        sm = tc.alloc_tile_pool(name="sm", bufs=3)
        psA = tc.alloc_tile_pool(name="psA", bufs=2, space="PSUM")
        psB = tc.alloc_tile_pool(name="psB", bufs=2, space="PSUM")
        psM = tc.alloc_tile_pool(name="psM", bufs=4, space="PSUM")

        # ---- constants ----
        ident = cst.tile([P, P], f32)
        make_identity(nc, ident[:])
        iota = cst.tile([P, P], f32, tag="iota")
        nc.sync.dma_start(out=iota[:], in_=iota_in[:])
        wi_t = cst.tile([D, D], f32, tag="wi")
        nc.sync.dma_start(out=wi_t[:], in_=Wi[:])
        wl_t = cst.tile([D, D], f32, tag="wl")
        nc.sync.dma_start(out=wl_t[:], in_=Wl[:])
        wres_t = cst.tile([D, D], f32, tag="wres")
        nc.sync.dma_start(out=wres_t[:], in_=Wres[:])
        wconv_t = [cst.tile([D, D], f32, name=f"wc{i}", tag=f"wc{i}") for i in range(3)]
        bi_t = cst.tile([P, D], f32, tag="bi")
        nc.sync.dma_start(out=bi_t[:], in_=bi_rep[:])
        bl_t = cst.tile([P, D], f32, tag="bl")
        nc.sync.dma_start(out=bl_t[:], in_=bl_rep[:])
        bc_t = [cst.tile([P, D], f32, name=f"bc{i}", tag=f"bc{i}") for i in range(3)]
        lg_t = [cst.tile([P, D], f32, name=f"lg{i}", tag=f"lg{i}") for i in range(3)]
        lb_t = [cst.tile([P, D], f32, name=f"lb{i}", tag=f"lb{i}") for i in range(3)]
        for i in range(3):
            nc.sync.dma_start(out=wconv_t[i][:], in_=convW[i])
            nc.sync.dma_start(out=bc_t[i][:], in_=convb_rep[i])
            nc.sync.dma_start(out=lg_t[i][:], in_=lng_rep[i])
            nc.sync.dma_start(out=lb_t[i][:], in_=lnb_rep[i])
        dinv_t = cst.tile([P, TNB], f32, tag="dinv")
        nc.sync.dma_start(out=dinv_t[:], in_=dinv_lay[:])
        eps_t = cst.tile([P, 1], f32, tag="eps")
        nc.vector.memset(eps_t[:], LN_EPS)

        # ---- persistent big tiles ----
        g_loc = big.tile([P, TNB * D], f32, tag="g_loc")
        agg = big.tile([P, TNB * D], f32, tag="agg")
        z = big.tile([P, TNB * D], f32, tag="z")
        racc = big.tile([P, TNB * D], f32, tag="racc")
        stats = big.tile([P, TNB * 8], f32, tag="stats")
        mean_t = big.tile([P, TNB], f32, tag="mean")
        d_t = big.tile([P, TNB], f32, tag="d")
        v_t = big.tile([P, TNB], f32, tag="v")
        rstd_t = big.tile([P, TNB], f32, tag="rstd")

        replica_groups = [list(range(NCORES))]

        def col(t):
            return slice(t * D, (t + 1) * D)

        def edge_phase(side, window_fn, idx_dram, dst_dram, val_dram, add_fn, name):
            """Gather + one-hot matmul scatter over one edge side."""
            idx_off = 0
            dst_off = 0
            open_psum = {}
            for w, runs in side["batches"]:
                C = sum(t for (_, t, _, _) in runs)
                NI = C * P
                idx = sm.tile([P, NI // 16], dt.int16, name=f"{name}_idx", tag="eg_idx")
                nc.sync.dma_start(
                    out=idx[:],
                    in_=idx_dram[idx_off:idx_off + P * (NI // 16)]
                    .rearrange("(p x) -> p x", p=P),
                )
                idx_off += P * (NI // 16)
                dstf = sm.tile([P, C], f32, name=f"{name}_dst", tag="eg_dst")
                nc.sync.dma_start(
                    out=dstf[:],
                    in_=dst_dram[dst_off:dst_off + P * C]
                    .rearrange("(p x) -> p x", p=P),
                )
                if val_dram is not None:
                    val = sm.tile([P, C], f32, name=f"{name}_val", tag="eg_val")
                    nc.sync.dma_start(
                        out=val[:],
                        in_=val_dram[dst_off:dst_off + P * C]
                        .rearrange("(p x) -> p x", p=P),
                    )
                dst_off += P * C

                gt = gat.tile([P, C * D], f32, name=f"{name}_gt", tag="eg_gt")
                nc.gpsimd.dma_gather(
                    out_ap=gt[:].rearrange("p (c d) -> p c d", d=D),
                    in_ap=window_fn(w),
                    idxs_ap=idx[:],
                    num_idxs=NI,
                    num_idxs_reg=NI,
                    elem_size=D,
                    single_packet=False,
                )
                if val_dram is not None:
                    nc.vector.tensor_tensor(
                        out=gt[:].rearrange("p (c d) -> p c d", d=D),
                        in0=gt[:].rearrange("p (c d) -> p c d", d=D),
                        in1=val[:].rearrange("p (c o) -> p c o", o=1)
                        .to_broadcast([P, C, D]),
                        op=Alu.mult,
                    )

                st = sm.tile([P, C * P], f32, name=f"{name}_st", tag="eg_st", bufs=2)
                for c0 in range(0, C, ST_BATCH):
                    cn = min(ST_BATCH, C - c0)
                    nc.vector.tensor_tensor(
                        out=st[:, c0 * P:(c0 + cn) * P]
                        .rearrange("p (n f) -> p n f", f=P),
                        in0=iota[:].rearrange("p (o f) -> p o f", o=1)
                        .to_broadcast([P, cn, P]),
                        in1=dstf[:, c0:c0 + cn].rearrange("p (n o) -> p n o", o=1)
                        .to_broadcast([P, cn, P]),
                        op=Alu.is_equal,
                    )

                c = 0
                for (b, take, first, last) in runs:
                    if first:
                        open_psum[b] = psM.tile([P, D], f32, name=f"{name}_ps", tag="eg_ps")
                    ps = open_psum[b]
                    for j in range(take):
                        nc.tensor.matmul(
                            out=ps[:],
                            lhsT=st[:, (c + j) * P:(c + j + 1) * P],
                            rhs=gt[:, (c + j) * D:(c + j + 1) * D],
                            start=(first and j == 0),
                            stop=(last and j == take - 1),
                        )
                    c += take
                    if last:
                        add_fn(b, open_psum.pop(b))
            assert not open_psum

        def h_chain(src_tile, t, w_t, bias_t, out_tile, out_slice, act_scale, relu):
            trp = psA.tile([D, P], f32, name="trp", tag="trp")
            nc.tensor.transpose(out=trp[:], in_=src_tile[:, col(t)], identity=ident[:])
            trs = sm.tile([D, P], f32, name="trs", tag="trs")
            nc.scalar.copy(out=trs[:], in_=trp[:])
            mm = psB.tile([P, D], f32, name="mm", tag="mm")
            nc.tensor.matmul(out=mm[:], lhsT=trs[:], rhs=w_t[:], start=True, stop=True)
            if relu:
                zt_ = sm.tile([P, D], f32, name="zstage", tag="zstage")
                nc.vector.scalar_tensor_tensor(
                    out=zt_[:], in0=mm[:], scalar=0.0, in1=bias_t[:],
                    op0=Alu.bypass, op1=Alu.add,
                )
                nc.scalar.activation(
                    out=out_tile[:, out_slice], in_=zt_[:], func=Act.Relu,
                    scale=act_scale if act_scale is not None else 1.0,
                )
            elif bias_t is not None:
                nc.vector.scalar_tensor_tensor(
                    out=out_tile[:, out_slice], in0=mm[:], scalar=0.0, in1=bias_t[:],
                    op0=Alu.bypass, op1=Alu.add,
                )
            else:
                nc.scalar.copy(out=out_tile[:, out_slice], in_=mm[:])

        # ---- stage 0: h0/g0 from x ----
        nc.sync.dma_start(out=agg[:], in_=x_lay[:])
        for t in range(TNB):
            h_chain(agg, t, wi_t, bi_t, g_loc, col(t), dinv_t[:, t:t + 1], relu=True)

        def push_g_and_allgather():
            nc.sync.dma_start(out=g_in[:].rearrange("(p x) -> p x", p=P), in_=g_loc[:])
            nc.gpsimd.collective_compute(
                "AllGather",
                Alu.bypass,
                ins=[g_in[:]],
                outs=[g_table[:]],
                replica_groups=replica_groups,
            )

        push_g_and_allgather()

        nc.vector.memset(racc[:], 0.0)

        def conv_add(b, ps):
            nc.vector.tensor_tensor(
                out=agg[:, col(b)], in0=agg[:, col(b)], in1=ps[:], op=Alu.add)

        def res_add(b, ps):
            nc.vector.tensor_tensor(
                out=racc[:, col(b)], in0=racc[:, col(b)], in1=ps[:], op=Alu.add)

        def conv_window(w):
            w0 = w * WIN
            return g_rows[w0:w0 + min(WIN, GROWS - w0), :]

        def res_window(w):
            w0 = w * WIN
            return x_org[w0:w0 + min(WIN, N - w0), :]

        # ---- conv layers ----
        for li in range(3):
            nc.vector.memset(agg[:], 0.0)
            edge_phase(meta["conv"], conv_window, conv_idx, conv_dst, None,
                       conv_add, f"cg{li}")

            # node phase: s = agg + g_loc ; u = s * dinv
            nc.vector.tensor_tensor(out=agg[:], in0=agg[:], in1=g_loc[:], op=Alu.add)
            nc.vector.tensor_tensor(
                out=agg[:].rearrange("p (t d) -> p t d", d=D),
                in0=agg[:].rearrange("p (t d) -> p t d", d=D),
                in1=dinv_t[:].rearrange("p (t o) -> p t o", o=1)
                .to_broadcast([P, TNB, D]),
                op=Alu.mult,
            )
            for t in range(TNB):
                h_chain(agg, t, wconv_t[li], bc_t[li], z, col(t), None, relu=False)
            for t in range(TNB):
                nc.vector.bn_stats(out=stats[:, t * 8:t * 8 + 6], in_=z[:, col(t)])
            sv = stats[:].rearrange("p (t s) -> p t s", s=8)
            me, m2e, mo, m2o = sv[:, :, 1], sv[:, :, 2], sv[:, :, 4], sv[:, :, 5]
            nc.vector.tensor_tensor(out=mean_t[:], in0=me, in1=mo, op=Alu.add)
            nc.vector.tensor_scalar_mul(mean_t[:], mean_t[:], 0.5)
            nc.vector.tensor_tensor(out=d_t[:], in0=me, in1=mo, op=Alu.subtract)
            nc.vector.tensor_tensor(out=v_t[:], in0=m2e, in1=m2o, op=Alu.add)
            nc.vector.tensor_tensor(out=d_t[:], in0=d_t[:], in1=d_t[:], op=Alu.mult)
            nc.vector.scalar_tensor_tensor(
                out=v_t[:], in0=d_t[:], scalar=16.0, in1=v_t[:],
                op0=Alu.mult, op1=Alu.add,
            )
            nc.scalar.activation(
                out=rstd_t[:], in_=v_t[:], func=Act.Sqrt, scale=1.0 / D, bias=eps_t[:]
            )
            nc.vector.reciprocal(rstd_t[:], rstd_t[:])
            last = li == 2
            for t in range(TNB):
                nc.vector.scalar_tensor_tensor(
                    out=z[:, col(t)], in0=z[:, col(t)], scalar=mean_t[:, t:t + 1],
                    in1=lg_t[li][:], op0=Alu.subtract, op1=Alu.mult,
                )
                nc.vector.scalar_tensor_tensor(
                    out=z[:, col(t)], in0=z[:, col(t)], scalar=rstd_t[:, t:t + 1],
                    in1=lb_t[li][:], op0=Alu.mult, op1=Alu.add,
                )
                nc.scalar.activation(
                    out=g_loc[:, col(t)], in_=z[:, col(t)], func=Act.Relu,
                    scale=1.0 if last else dinv_t[:, t:t + 1],
                )
            if not last:
                push_g_and_allgather()

        # ---- final linear: out = h3 @ Wl + bl ----
        for t in range(TNB):
            h_chain(g_loc, t, wl_t, bl_t, z, col(t), None, relu=False)
        nc.sync.dma_start(out=out_sh[:], in_=z[:])

        # ---- residual: (sum val * x_org[dst]) @ Wres ----
        edge_phase(meta["res"], res_window, res_idx, res_dst, res_val,
                   res_add, "rg")
        for t in range(TNB):
            h_chain(racc, t, wres_t, None, agg, col(t), None, relu=False)
        nc.sync.dma_start(out=res_sh[:], in_=agg[:])

        for pool in (psM, psB, psA, sm, gat, big, cst):
            pool.release()

    nc.compile()
    return nc


# ----------------------------------------------------------------------------
# Entry point
# ----------------------------------------------------------------------------

def _in_maps(meta, inputs):
    rep = lambda v: np.broadcast_to(np.asarray(v, np.float32), (P, D)).copy()
    rep3 = lambda v: np.stack([rep(np.asarray(v)[i]) for i in range(3)])
    common = dict(
        x_org=np.asarray(inputs["x_org"], np.float32),
        iota_in=np.tile(np.arange(P, dtype=np.float32), (P, 1)).copy(),
        Wi=np.asarray(inputs["Wi"], np.float32),
        convW=np.asarray(inputs["conv_W"], np.float32),
        Wl=np.asarray(inputs["Wl"], np.float32),
        Wres=np.asarray(inputs["Wres"], np.float32),
        bi_rep=rep(inputs["bi"]),
        bl_rep=rep(inputs["bl"]),
        convb_rep=rep3(inputs["conv_b"]),
        lng_rep=rep3(inputs["ln_g"]),
        lnb_rep=rep3(inputs["ln_b"]),
    )
    maps = []
    for c in range(NCORES):
        m = dict(common)
        m["x_lay"] = meta["x_lay"][c]
        m["dinv_lay"] = meta["dinv_lay"][c]
        m["conv_idx"] = meta["conv"]["idx"][c]
        m["conv_dst"] = meta["conv"]["dst"][c]
        m["res_idx"] = meta["res"]["idx"][c]
        m["res_dst"] = meta["res"]["dst"][c]
        m["res_val"] = meta["res"]["val"][c]
        maps.append(m)
    return maps


def _assemble(meta, results):
    N, PER, TNB = meta["N"], meta["PER"], meta["TNB"]
    out = np.zeros((N, D), np.float32)
    residual = np.zeros((N, D), np.float32)
    for c in range(NCORES):
        o = results[c]["out_sh"].reshape(P, TNB, D).transpose(1, 0, 2)
        out[c * PER:(c + 1) * PER] = o.reshape(TNB * P, D)[:PER]
        r = results[c]["res_sh"].reshape(P, TNB, D).transpose(1, 0, 2)
        residual[c * PER:(c + 1) * PER] = r.reshape(TNB * P, D)[:PER]
    return out, residual


def kernel(x, x_org, adj_values, Wi, bi, conv_W, conv_b, ln_g, ln_b, Wl, bl, Wres,
           edge_index):
    inputs = dict(x=x, x_org=x_org, adj_values=adj_values, Wi=Wi, bi=bi,
                  conv_W=conv_W, conv_b=conv_b, ln_g=ln_g, ln_b=ln_b, Wl=Wl,
                  bl=bl, Wres=Wres)
    meta = _preprocess(x, x_org, adj_values, edge_index)
    nc = _build_bass(meta)

    from concourse.bass_utils import run_bass_kernel_spmd

    trace = os.environ.get("GCN_TRACE", "0") == "1"
    res = run_bass_kernel_spmd(
        nc, _in_maps(meta, inputs), core_ids=list(range(NCORES)), trace=trace,
        trace_kwargs={"title": "gcn_encoder"} if trace else {},
    )
    if trace and res.exec_time_ns is not None:
        print(f"HW exec time: {res.exec_time_ns} ns")
        if res.instructions_and_trace:
            print(f"trace: {res.instructions_and_trace[1]}")
    return _assemble(meta, res.results)



# revision 14
# speedup vs baseline: 1.1980x; 1.1980x over previous
"""GCN encoder (3-layer GCNConv + LN + relu, plus sparse residual) on 8 trn2 NeuronCores.

Design notes (v2), driven by perfetto traces:
  - The hard floor is SWDGE descriptor generation on the GpSimd Q7 pair
    (~7.5ns per gathered row), so only the 3 conv layers use dma_gather;
    everything else is kept off the Pool engine and overlapped under it.
  - Nodes sharded by contiguous id range; conv edges owned by dst core.
  - Algebra: the gathered table holds q = g*dinv in bf16 (256B rows, 64
    payload + 64 pad), so conv edges carry no per-edge weight; self-loop and
    dst-side dinv fold into the node phase.
  - Conv scatter: one-hot matmuls (bf16) accumulate feature-major PSUM cells
    [64, 512] (4 dst-blocks); cells are ordered in 2-group segments so each
    group's node phase interleaves with the next segment's gathers.
  - Residual: the gather source (x_org[dst]) is static, so the full per-edge
    row stream is laid out host-side (bf16) and streamed with affine DMA:
    zero SWDGE. adj_values are applied to the payload with one batched
    multiply. Residual batches are spliced between conv batches to fill
    non-Q7 engines while gathers run.
  - Node phase is feature-major: weight matmuls stream [64, 512] columns, LN
    stats via ones-matmul partition reduction, per-node broadcasts via k=1
    outer-product matmuls, rstd via Ln/Exp on the Scalar engine. Outputs stay
    feature-major; the host transposes on assembly.
  - Per layer each core transposes its q rows to node-major via PE and an
    AllGather replicates the table for the next layer's gathers.
"""

import os

import numpy as np

P = 128
D = 64
NCORES = 8
WIN = 32768          # dma_gather int16 index window (table rows)
GW = 512             # dst columns per PSUM cell (4 blocks of 128)
SEG = 2              # groups per conv segment (open PSUM accumulators)
C_BUDGET = 48        # chunks (of 128 edges) per gather batch
ST_BATCH = 8         # chunks per one-hot build DVE op
LN_EPS = 1e-5
PAD_DST = 999        # pad dst value (no iota512 column matches)


# ----------------------------------------------------------------------------
# Host-side preprocessing
# ----------------------------------------------------------------------------

def _make_conv_batches(chunks_eq):
    """Segment-ordered conv batches with group-level first/last flags.

    Cell order: for each segment of SEG groups, for each window, the segment's
    groups. Batches never span windows. Returns (batches, seg_ends) where
    seg_ends[i] = index into batches after which segment i's groups have all
    closed."""
    NW, NG = chunks_eq.shape
    totals = chunks_eq.sum(0)
    emitted = np.zeros(NG, np.int64)
    batches = []
    cur, room, cur_w = [], C_BUDGET, None

    def flush():
        nonlocal cur, room, cur_w
        if cur:
            batches.append((cur_w, cur))
        cur, room, cur_w = [], C_BUDGET, None

    for s0 in range(0, NG, SEG):
        gs = list(range(s0, min(s0 + SEG, NG)))
        for w in range(NW):
            for g in gs:
                n = int(chunks_eq[w, g])
                while n > 0:
                    if cur_w is not None and cur_w != w:
                        flush()
                    cur_w = w
                    take = min(n, room)
                    first = emitted[g] == 0
                    emitted[g] += take
                    cur.append((g, take, first, emitted[g] == totals[g]))
                    n -= take
                    room -= take
                    if room == 0:
                        flush()
        flush()
    return batches


def _make_res_batches(chunks_eq):
    """Group-ordered residual batches (single pseudo-window)."""
    NG = chunks_eq.shape[1]
    batches = []
    cur, room = [], C_BUDGET
    for g in range(NG):
        n = int(chunks_eq[0, g])
        first = True
        while n > 0:
            take = min(n, room)
            cur.append((g, take, first, n - take == 0))
            first = False
            n -= take
            room -= take
            if room == 0:
                batches.append((0, cur))
                cur, room = [], C_BUDGET
    if cur:
        batches.append((0, cur))
    return batches


def _pack_core(batches, counts, starts, idx16, dwg, val, rows):
    """Emit one core's flat arrays in batch order."""
    consumed = {}
    idx_parts, dst_parts, val_parts, row_parts = [], [], [], []
    for w, runs in batches:
        bi, bd, bv, br = [], [], [], []
        for (g, take, first, last) in runs:
            done = consumed.get((w, g), 0)
            s = int(starts[w, g]) + done * P
            e = min(int(starts[w, g]) + int(counts[w, g]), s + take * P)
            n_real = max(0, e - s)
            dd = np.full(take * P, PAD_DST, np.float32)
            dd[:n_real] = dwg[s:s + n_real]
            bd.append(dd)
            if idx16 is not None:
                ii = np.zeros(take * P, np.int16)
                ii[:n_real] = idx16[s:s + n_real]
                bi.append(ii)
            if val is not None:
                vv = np.zeros(take * P, np.float32)
                vv[:n_real] = val[s:s + n_real]
                bv.append(vv)
            if rows is not None:
                rr = np.zeros((take * P, D), np.float32)
                rr[:n_real] = rows[s:s + n_real]
                br.append(rr)
            consumed[(w, g)] = done + take
        flat = np.concatenate(bd)
        dst_parts.append(np.ascontiguousarray(flat.reshape(-1, P).T).ravel())
        if idx16 is not None:
            flat = np.concatenate(bi)
            NI = len(flat)
            a = flat.reshape(NI // 16, 16).T
            idx_parts.append(np.ascontiguousarray(np.tile(a, (8, 1))).ravel())
        if val is not None:
            flat = np.concatenate(bv)
            val_parts.append(np.ascontiguousarray(flat.reshape(-1, P).T).ravel())
        if rows is not None:
            flat = np.concatenate(br).reshape(-1, P, D)
            row_parts.append(np.ascontiguousarray(flat.transpose(1, 0, 2)).ravel())
    import ml_dtypes
    return dict(
        idx=np.concatenate(idx_parts) if idx_parts else None,
        dst=np.concatenate(dst_parts),
        val=np.concatenate(val_parts) if val_parts else None,
        rows=(np.concatenate(row_parts).astype(ml_dtypes.bfloat16)
              if row_parts else None),
    )


def _merge_program(conv_batches, res_slab):
    """Splice residual batches between conv batches, evenly spread."""
    prog = []
    if not res_slab:
        return [("conv", b) for b in conv_batches]
    k = max(1, len(conv_batches) // (len(res_slab) + 1))
    ri = 0
    for i, b in enumerate(conv_batches):
        prog.append(("conv", b))
        if (i + 1) % k == 0 and ri < len(res_slab):
            prog.append(("res", res_slab[ri]))
            ri += 1
    prog.extend(("res", b) for b in res_slab[ri:])
    return prog


def _preprocess(x, x_org, adj_values, edge_index):
    N = x.shape[0]
    assert N % NCORES == 0
    PER = N // NCORES
    PAD_N = -(-PER // GW) * GW
    NG = PAD_N // GW
    GROWS = NCORES * PAD_N
    NW = -(-GROWS // WIN)
    src = np.asarray(edge_index[0], dtype=np.int64)
    dst = np.asarray(edge_index[1], dtype=np.int64)
    adj_values = np.asarray(adj_values, dtype=np.float32)
    x_org = np.asarray(x_org, np.float32)

    deg_in = np.bincount(dst, minlength=N)
    dinv = (1.0 / np.sqrt(deg_in + 1.0)).astype(np.float32)

    gid = (np.arange(N) // PER) * PAD_N + (np.arange(N) % PER)

    # ---- conv side: edges by dst owner, cells (window, dst 512-group)
    conv_cores = []
    conv_counts = np.zeros((NCORES, NW, NG), np.int64)
    for c in range(NCORES):
        m = (dst >= c * PER) & (dst < (c + 1) * PER)
        sg = gid[src[m]]
        w = sg // WIN
        dl = dst[m] - c * PER
        g = dl // GW
        order = np.lexsort((dl, w, g // SEG))  # segment-major, then window
        wo = w[order]
        conv_cores.append(dict(
            idx16=(sg[order] - wo * WIN).astype(np.int16),
            dwg=(dl % GW)[order].astype(np.float32),
        ))
        conv_counts[c] = np.bincount(w * NG + g, minlength=NW * NG).reshape(NW, NG)
    conv_eq = -(-conv_counts.max(0) // P)
    conv_eq[0] = np.maximum(conv_eq[0], 1)  # every group closes at least once
    conv_batches = _make_conv_batches(conv_eq)

    # ---- residual side: edges by src owner, sorted by (src 512-group);
    # payload rows x_org[dst] are static -> host-laid-out affine stream.
    res_cores = []
    res_counts = np.zeros((NCORES, 1, NG), np.int64)
    for c in range(NCORES):
        m = (src >= c * PER) & (src < (c + 1) * PER)
        sl = src[m] - c * PER
        g = sl // GW
        order = np.argsort(g, kind="stable")
        res_cores.append(dict(
            dwg=(sl % GW)[order].astype(np.float32),
            val=adj_values[m][order],
            dstrows=dst[m][order],
        ))
        res_counts[c, 0] = np.bincount(g, minlength=NG)
    res_eq = -(-res_counts.max(0) // P)
    res_eq[0] = np.maximum(res_eq[0], 1)
    res_batches = _make_res_batches(res_eq)

    conv_packed, res_packed = [], []
    for c in range(NCORES):
        cc = conv_cores[c]
        cnt = conv_counts[c]
        # starts must follow the SORTED edge order: cumulative position of
        # each (w, g) cell in segment-major, window, group order
        cell_starts = np.zeros((NW, NG), np.int64)
        pos = 0
        for s0 in range(0, NG, SEG):
            for w in range(NW):
                for g in range(s0, min(s0 + SEG, NG)):
                    cell_starts[w, g] = pos
                    pos += int(cnt[w, g])
        conv_packed.append(_pack_core(
            conv_batches, cnt, cell_starts,
            cc["idx16"], cc["dwg"], None, None))
        rc = res_cores[c]
        cnt = res_counts[c]
        starts = np.zeros(NG + 1, np.int64)
        np.cumsum(cnt.ravel(), out=starts[1:])
        res_packed.append(_pack_core(
            res_batches, cnt, starts[:-1].reshape(1, NG),
            None, rc["dwg"], rc["val"], x_org[rc["dstrows"]]))

    # ---- feature-major per-core inputs
    import ml_dtypes
    x = np.asarray(x, np.float32)
    x_fm, d_fm = [], []
    for c in range(NCORES):
        xm = np.zeros((D, PAD_N), np.float32)
        xm[:, :PER] = x[c * PER:(c + 1) * PER].T
        x_fm.append(xm)
        dm = np.zeros((1, PAD_N), np.float32)
        dm[0, :PER] = dinv[c * PER:(c + 1) * PER]
        d_fm.append(np.broadcast_to(dm, (D, PAD_N)).astype(ml_dtypes.bfloat16))

    return dict(
        N=N, PER=PER, PAD_N=PAD_N, NG=NG, GROWS=GROWS, NW=NW,
        conv_batches=conv_batches, res_batches=res_batches,
        conv=conv_packed, res=res_packed,
        x_fm=x_fm, d_fm=d_fm,
    )


# ----------------------------------------------------------------------------
# Bass kernel builder
# ----------------------------------------------------------------------------

def _build_bass(meta):
    import concourse.bacc as bacc
    import concourse.bass as bass  # noqa: F401
    import concourse.mybir as mybir
    import concourse.tile as tile
    from concourse.masks import make_identity

    dt = mybir.dt
    Alu = mybir.AluOpType
    Act = mybir.ActivationFunctionType
    f32 = dt.float32
    bf16 = dt.bfloat16

    PAD_N = meta["PAD_N"]
    NG = meta["NG"]
    GROWS = meta["GROWS"]
    TW = 2 * D  # table row width in bf16 elements (256B rows)

    nc = bacc.Bacc(
        "TRN2",
        target_bir_lowering=False,
        debug=False,
        enable_asserts=False,
        num_devices=NCORES,
    )

    # ---- I/O ----
    conv_idx = nc.dram_tensor("conv_idx", [len(meta["conv"][0]["idx"])], dt.int16,
                              kind="ExternalInput")
    conv_dst = nc.dram_tensor("conv_dst", [len(meta["conv"][0]["dst"])], f32,
                              kind="ExternalInput")
    res_dst = nc.dram_tensor("res_dst", [len(meta["res"][0]["dst"])], f32,
                             kind="ExternalInput")
    res_val = nc.dram_tensor("res_val", [len(meta["res"][0]["val"])], f32,
                             kind="ExternalInput")
    res_rows = nc.dram_tensor("res_rows", [len(meta["res"][0]["rows"])], bf16,
                              kind="ExternalInput")
    x_fm = nc.dram_tensor("x_fm", [D, PAD_N], f32, kind="ExternalInput")
    d_fm = nc.dram_tensor("d_fm", [D, PAD_N], bf16, kind="ExternalInput")
    iota_in = nc.dram_tensor("iota_in", [P, GW], f32, kind="ExternalInput")
    Wi = nc.dram_tensor("Wi", [D, D], f32, kind="ExternalInput")
    convW = nc.dram_tensor("convW", [3, D, D], f32, kind="ExternalInput")
    Wl = nc.dram_tensor("Wl", [D, D], f32, kind="ExternalInput")
    Wres = nc.dram_tensor("Wres", [D, D], f32, kind="ExternalInput")
    bi_col = nc.dram_tensor("bi_col", [D, 1], f32, kind="ExternalInput")
    bl_col = nc.dram_tensor("bl_col", [D, 1], f32, kind="ExternalInput")
    convb_col = nc.dram_tensor("convb_col", [3, D, 1], f32, kind="ExternalInput")
    lnb_col = nc.dram_tensor("lnb_col", [3, D, 1], f32, kind="ExternalInput")
    lng_row = nc.dram_tensor("lng_row", [3, 1, D], f32, kind="ExternalInput")
    mean_row = nc.dram_tensor("mean_row", [D, 1], f32, kind="ExternalInput")
    ones_row = nc.dram_tensor("ones_row", [1, D], f32, kind="ExternalInput")

    out_sh = nc.dram_tensor("out_sh", [D, PAD_N], f32, kind="ExternalOutput")
    res_sh = nc.dram_tensor("res_sh", [D, PAD_N], f32, kind="ExternalOutput")

    # ---- internal DRAM ----
    g_in = nc.dram_tensor("g_in", [PAD_N * TW], bf16)
    g_table = nc.dram_tensor("g_table", [GROWS * TW], bf16, addr_space="Shared")
    g_rows = g_table[:].rearrange("(r d) -> r d", d=TW)

    replica_groups = [list(range(NCORES))]

    with tile.TileContext(nc) as tc:
        cst = tc.alloc_tile_pool(name="cst", bufs=1)
        fmp = tc.alloc_tile_pool(name="fmp", bufs=1)
        gat = tc.alloc_tile_pool(name="gat", bufs=2)
        sm = tc.alloc_tile_pool(name="sm", bufs=2)
        nd = tc.alloc_tile_pool(name="nd", bufs=2)
        psA = tc.alloc_tile_pool(name="psA", bufs=3, space="PSUM")
        psR = tc.alloc_tile_pool(name="psR", bufs=1, space="PSUM")
        psB = tc.alloc_tile_pool(name="psB", bufs=2, space="PSUM")
        psT = tc.alloc_tile_pool(name="psT", bufs=1, space="PSUM")
        psS = tc.alloc_tile_pool(name="psS", bufs=1, space="PSUM")

        # ---- constants ----
        ident = cst.tile([D, D], f32)
        make_identity(nc, ident[:])
        iota = cst.tile([P, GW], f32, tag="iota")
        nc.sync.dma_start(out=iota[:], in_=iota_in[:])
        wi_t = cst.tile([D, D], f32, tag="wi")
        nc.sync.dma_start(out=wi_t[:], in_=Wi[:])
        wl_t = cst.tile([D, D], f32, tag="wl")
        nc.sync.dma_start(out=wl_t[:], in_=Wl[:])
        wres_t = cst.tile([D, D], f32, tag="wres")
        nc.sync.dma_start(out=wres_t[:], in_=Wres[:])
        wc_t = [cst.tile([D, D], f32, name=f"wc{i}", tag=f"wc{i}") for i in range(3)]
        bc_t = [cst.tile([D, 1], f32, name=f"bc{i}", tag=f"bc{i}") for i in range(3)]
        lg_t = [cst.tile([1, D], f32, name=f"lg{i}", tag=f"lg{i}") for i in range(3)]
        lb_t = [cst.tile([D, 1], f32, name=f"lb{i}", tag=f"lb{i}") for i in range(3)]
        for i in range(3):
            nc.sync.dma_start(out=wc_t[i][:], in_=convW[i])
            nc.sync.dma_start(out=bc_t[i][:], in_=convb_col[i])
            nc.sync.dma_start(out=lg_t[i][:], in_=lng_row[i])
            nc.sync.dma_start(out=lb_t[i][:], in_=lnb_col[i])
        bi_t = cst.tile([D, 1], f32, tag="bi")
        nc.sync.dma_start(out=bi_t[:], in_=bi_col[:])
        bl_t = cst.tile([D, 1], f32, tag="bl")
        nc.sync.dma_start(out=bl_t[:], in_=bl_col[:])
        mean_t = cst.tile([D, 1], f32, tag="mean")  # 1/64 column
        nc.sync.dma_start(out=mean_t[:], in_=mean_row[:])
        ones_t = cst.tile([1, D], f32, tag="ones")  # 1.0 row
        nc.sync.dma_start(out=ones_t[:], in_=ones_row[:])
        eps_t = cst.tile([1, 1], f32, tag="eps")
        nc.vector.memset(eps_t[:], LN_EPS)

        # ---- persistent feature-major tiles ----
        q_loc = fmp.tile([D, PAD_N], bf16, tag="q_loc")
        d_t = fmp.tile([D, PAD_N], bf16, tag="d_t")
        nc.sync.dma_start(out=d_t[:], in_=d_fm[:])

        def gcols(g):
            return slice(g * GW, (g + 1) * GW)

        # ------------------------------------------------------------------
        # Edge machinery
        # ------------------------------------------------------------------
        def conv_gather(w, C, state):
            gt = gat.tile([P, C_BUDGET * TW], bf16, name="cg_gt", tag="eg_gt_c")
            NI = C * P
            idx = sm.tile([P, C_BUDGET * 8], dt.int16, name="cg_idx", tag="eg_idx")
            nc.sync.dma_start(
                out=idx[:, :NI // 16],
                in_=conv_idx[state["idx"]:state["idx"] + P * (NI // 16)]
                .rearrange("(p x) -> p x", p=P),
            )
            state["idx"] += P * (NI // 16)
            w0 = w * WIN
            nc.gpsimd.dma_gather(
                out_ap=gt[:, :C * TW].rearrange("p (c d) -> p c d", d=TW),
                in_ap=g_rows[w0:w0 + min(WIN, GROWS - w0), :],
                idxs_ap=idx[:, :NI // 16],
                num_idxs=NI,
                num_idxs_reg=NI,
                elem_size=TW,
                single_packet=False,
            )
            return lambda ci: gt[:, ci * TW:ci * TW + D]

        def res_gather(w, C, state):
            gt = gat.tile([P, C_BUDGET * D], bf16, name="rs_gt", tag="eg_gt_r")
            nc.sync.dma_start(
                out=gt[:, :C * D],
                in_=res_rows[state["rows"]:state["rows"] + P * C * D]
                .rearrange("(p x) -> p x", p=P),
            )
            state["rows"] += P * C * D
            val = sm.tile([P, C_BUDGET], f32, name="rs_val", tag="eg_val")
            nc.sync.dma_start(
                out=val[:, :C],
                in_=res_val[state["val"]:state["val"] + P * C]
                .rearrange("(p x) -> p x", p=P),
            )
            state["val"] += P * C
            nc.vector.tensor_tensor(
                out=gt[:, :C * D].rearrange("p (c d) -> p c d", d=D),
                in0=gt[:, :C * D].rearrange("p (c d) -> p c d", d=D),
                in1=val[:, :C].rearrange("p (c o) -> p c o", o=1)
                .to_broadcast([P, C, D]),
                op=Alu.mult,
            )
            return lambda ci: gt[:, ci * D:(ci + 1) * D]

        def run_batch(w, runs, state, gather_fn, dst_dram, pool, tag, bufs,
                      open_psum, close_fn):
            C = sum(t for (_, t, _, _) in runs)
            dstf = sm.tile([P, C_BUDGET], f32, name=f"{tag}_dst", tag="eg_dst")
            nc.sync.dma_start(
                out=dstf[:, :C],
                in_=dst_dram[state["dst"]:state["dst"] + P * C]
                .rearrange("(p x) -> p x", p=P),
            )
            state["dst"] += P * C
            gt_ap = gather_fn(w, C, state)
            sts = []
            for c0 in range(0, C, ST_BATCH):
                cn = min(ST_BATCH, C - c0)
                st = sm.tile([P, ST_BATCH * GW], bf16, name=f"st{c0}",
                             tag="eg_st", bufs=3)
                nc.vector.tensor_tensor(
                    out=st[:, :cn * GW].rearrange("p (n f) -> p n f", f=GW),
                    in0=iota[:].rearrange("p (o f) -> p o f", o=1)
                    .to_broadcast([P, cn, GW]),
                    in1=dstf[:, c0:c0 + cn].rearrange("p (n o) -> p n o", o=1)
                    .to_broadcast([P, cn, GW]),
                    op=Alu.is_equal,
                )
                sts.append(st)
            c = 0
            for (g, take, first, last) in runs:
                if first:
                    open_psum[g] = pool.tile([D, GW], f32, name=f"{tag}_ps",
                                             tag=tag, bufs=bufs)
                ps = open_psum[g]
                for j in range(take):
                    ci = c + j
                    st = sts[ci // ST_BATCH]
                    so = (ci % ST_BATCH) * GW
                    nc.tensor.matmul(
                        out=ps[:],
                        lhsT=gt_ap(ci),
                        rhs=st[:, so:so + GW],
                        start=(first and j == 0),
                        stop=(last and j == take - 1),
                    )
                c += take
                if last:
                    close_fn(g, open_psum.pop(g))

        def res_close(g, ps):
            rb = nd.tile([D, GW], f32, name="res_rb", tag="res_rb")
            nc.scalar.copy(out=rb[:], in_=ps[:])
            rps = psB.tile([D, GW], f32, name="res_rps", tag="mm_out")
            nc.tensor.matmul(out=rps[:], lhsT=wres_t[:], rhs=rb[:],
                             start=True, stop=True)
            ro = nd.tile([D, GW], f32, name="res_ro", tag="res_ro")
            nc.scalar.copy(out=ro[:], in_=rps[:])
            nc.sync.dma_start(out=res_sh[:, gcols(g)], in_=ro[:])

        # ------------------------------------------------------------------
        # Node phase (feature-major, per 512-node group)
        # ------------------------------------------------------------------
        def push_rows(src_tile, g):
            """Transpose [64, 512] fm tile to node-major and write table rows."""
            stg = nd.tile([P, (GW // P) * TW], bf16, name="stg", tag="stg")
            nc.vector.memset(stg[:], 0.0)
            for k in range(GW // P):
                tp = psT.tile([P, D], f32, name="tp", tag="tp")
                nc.tensor.transpose(
                    out=tp[:], in_=src_tile[:, k * P:(k + 1) * P],
                    identity=ident[:])
                nc.scalar.copy(out=stg[:, k * TW:k * TW + D], in_=tp[:])
            nc.sync.dma_start(
                out=g_in[g * GW * TW:(g + 1) * GW * TW]
                .rearrange("(k p d) -> p k d", p=P, d=TW),
                in_=stg[:].rearrange("p (k d) -> p k d", d=TW),
            )

        def lin_head(rhs_ap, w_tile, b_tile, relu, name):
            """z = W.T @ rhs (+bias); returns SBUF tile [64, GW]."""
            zps = psB.tile([D, GW], f32, name=f"{name}_zps", tag="mm_out")
            nc.tensor.matmul(out=zps[:], lhsT=w_tile[:], rhs=rhs_ap,
                             start=True, stop=True)
            z = nd.tile([D, GW], f32, name=f"{name}_z", tag="nd_z")
            if relu:
                nc.scalar.activation(out=z[:], in_=zps[:], func=Act.Relu,
                                     scale=1.0, bias=b_tile[:])
            else:
                nc.vector.tensor_tensor(
                    out=z[:], in0=zps[:],
                    in1=b_tile[:].to_broadcast([D, GW]), op=Alu.add)
            return z

        def h0_group(g):
            xt = nd.tile([D, GW], f32, name="h0_x", tag="nd_x")
            nc.sync.dma_start(out=xt[:], in_=x_fm[:, gcols(g)])
            z = lin_head(xt[:], wi_t, bi_t, relu=True, name="h0")
            qn = nd.tile([D, GW], f32, name="h0_qn", tag="nd_qn")
            nc.vector.tensor_tensor(out=qn[:], in0=z[:], in1=d_t[:, gcols(g)],
                                    op=Alu.mult)
            nc.scalar.copy(out=q_loc[:, gcols(g)], in_=qn[:])
            push_rows(qn, g)

        def conv_group(g, li, last, ps):
            # pre = (agg + q_loc) * dinv, straight from the scatter PSUM
            t = nd.tile([D, GW], f32, name="cg_t", tag="nd_t")
            nc.vector.tensor_tensor(out=t[:], in0=ps[:],
                                    in1=q_loc[:, gcols(g)], op=Alu.add)
            nc.vector.tensor_tensor(out=t[:], in0=t[:], in1=d_t[:, gcols(g)],
                                    op=Alu.mult)
            z = lin_head(t[:], wc_t[li], bc_t[li], relu=False, name="cg")
            # LN stats: mu = (1/64) . z ; msq = (1/64) . z^2
            sq = nd.tile([D, GW], f32, name="cg_sq", tag="nd_sq")
            nc.vector.tensor_tensor(out=sq[:], in0=z[:], in1=z[:], op=Alu.mult)
            mu_ps = psS.tile([1, GW], f32, name="cg_mups", tag="st_ps")
            nc.tensor.matmul(out=mu_ps[:], lhsT=mean_t[:], rhs=z[:],
                             start=True, stop=True)
            mu = nd.tile([1, GW], f32, name="cg_mu", tag="st_mu_sb", bufs=1)
            nc.scalar.copy(out=mu[:], in_=mu_ps[:])
            ms_ps = psS.tile([1, GW], f32, name="cg_msps", tag="st_ps")
            nc.tensor.matmul(out=ms_ps[:], lhsT=mean_t[:], rhs=sq[:],
                             start=True, stop=True)
            var = nd.tile([1, GW], f32, name="cg_var", tag="st_var", bufs=1)
            nc.vector.tensor_tensor(out=var[:], in0=mu[:], in1=mu[:], op=Alu.mult)
            nc.vector.tensor_tensor(out=var[:], in0=ms_ps[:], in1=var[:],
                                    op=Alu.subtract)
            # rstd = exp(-0.5 * ln(var + eps)) on the Scalar engine
            lv = nd.tile([1, GW], f32, name="cg_lv", tag="st_lv", bufs=1)
            nc.scalar.activation(out=lv[:], in_=var[:], func=Act.Ln,
                                 scale=1.0, bias=eps_t[:])
            rstd = nd.tile([1, GW], f32, name="cg_rstd", tag="st_rstd", bufs=1)
            nc.scalar.activation(out=rstd[:], in_=lv[:], func=Act.Exp,
                                 scale=-0.5)
            # broadcasts: A = gamma (x) rstd ; mu_b = ones (x) mu
            a_ps = psB.tile([D, GW], f32, name="cg_aps", tag="mm_out")
            nc.tensor.matmul(out=a_ps[:], lhsT=lg_t[li][:], rhs=rstd[:],
                             start=True, stop=True)
            mb_ps = psB.tile([D, GW], f32, name="cg_mbps", tag="mm_out")
            nc.tensor.matmul(out=mb_ps[:], lhsT=ones_t[:], rhs=mu[:],
                             start=True, stop=True)
            t1 = nd.tile([D, GW], f32, name="cg_t1", tag="nd_t")
            nc.vector.tensor_tensor(out=t1[:], in0=z[:], in1=mb_ps[:],
                                    op=Alu.subtract)
            t2 = nd.tile([D, GW], f32, name="cg_t2", tag="nd_sq")
            nc.vector.tensor_tensor(out=t2[:], in0=t1[:], in1=a_ps[:],
                                    op=Alu.mult)
            h = nd.tile([D, GW], f32, name="cg_h", tag="nd_h")
            nc.scalar.activation(out=h[:], in_=t2[:], func=Act.Relu,
                                 scale=1.0, bias=lb_t[li][:])
            if not last:
                qn = nd.tile([D, GW], f32, name="cg_qn", tag="nd_qn")
                nc.vector.tensor_tensor(out=qn[:], in0=h[:], in1=d_t[:, gcols(g)],
                                        op=Alu.mult)
                nc.scalar.copy(out=q_loc[:, gcols(g)], in_=qn[:])
                push_rows(qn, g)
            else:
                o = lin_head(h[:], wl_t, bl_t, relu=False, name="fin")
                nc.sync.dma_start(out=out_sh[:, gcols(g)], in_=o[:])

        def allgather():
            nc.gpsimd.collective_compute(
                "AllGather",
                Alu.bypass,
                ins=[g_in[:]],
                outs=[g_table[:]],
                replica_groups=replica_groups,
            )

        # ------------------------------------------------------------------
        # Program
        # ------------------------------------------------------------------
        for g in range(NG):
            h0_group(g)
        allgather()

        # residual batches split into 3 slabs at group boundaries
        rb = meta["res_batches"]
        clean = [i + 1 for i, (w, runs) in enumerate(rb) if runs[-1][3]]

        def nearest(t):
            return min(clean, key=lambda i: abs(i - t))

        b1, b2 = nearest(len(rb) // 3), nearest(2 * len(rb) // 3)
        res_slabs = [rb[:b1], rb[b1:b2], rb[b2:]]
        res_state = dict(rows=0, dst=0, val=0)
        open_res = {}

        for li in range(3):
            last = li == 2
            conv_state = dict(idx=0, dst=0)
            open_conv = {}
            prog = _merge_program(meta["conv_batches"], res_slabs[li])

            def conv_close(g, ps, _li=li, _last=last):
                conv_group(g, _li, _last, ps)

            for kind, (w, runs) in prog:
                if kind == "conv":
                    run_batch(w, runs, conv_state, conv_gather, conv_dst,
                              psA, "eg_ps", 3, open_conv, conv_close)
                else:
                    run_batch(w, runs, res_state, res_gather, res_dst,
                              psR, "res_ps", 1, open_res, res_close)
            assert not open_conv
            if not last:
                allgather()
        assert not open_res

        for pool in (psS, psT, psB, psR, psA, nd, sm, gat, fmp, cst):
            pool.release()

    nc.compile()
    return nc


# ----------------------------------------------------------------------------
# Entry point
# ----------------------------------------------------------------------------

def _in_maps(meta, inputs):
    col = lambda v: np.asarray(v, np.float32).reshape(D, 1).copy()
    col3 = lambda v: np.stack([col(np.asarray(v)[i]) for i in range(3)])
    row3 = lambda v: np.stack(
        [np.asarray(v, np.float32)[i].reshape(1, D).copy() for i in range(3)])
    common = dict(
        iota_in=np.tile(np.arange(GW, dtype=np.float32), (P, 1)).copy(),
        Wi=np.asarray(inputs["Wi"], np.float32),
        convW=np.asarray(inputs["conv_W"], np.float32),
        Wl=np.asarray(inputs["Wl"], np.float32),
        Wres=np.asarray(inputs["Wres"], np.float32),
        bi_col=col(inputs["bi"]),
        bl_col=col(inputs["bl"]),
        convb_col=col3(inputs["conv_b"]),
        lnb_col=col3(inputs["ln_b"]),
        lng_row=row3(inputs["ln_g"]),
        mean_row=np.full((D, 1), 1.0 / D, np.float32),
        ones_row=np.ones((1, D), np.float32),
    )
    maps = []
    for c in range(NCORES):
        m = dict(common)
        m["x_fm"] = meta["x_fm"][c]
        m["d_fm"] = meta["d_fm"][c]
        m["conv_idx"] = meta["conv"][c]["idx"]
        m["conv_dst"] = meta["conv"][c]["dst"]
        m["res_dst"] = meta["res"][c]["dst"]
        m["res_val"] = meta["res"][c]["val"]
        m["res_rows"] = meta["res"][c]["rows"]
        maps.append(m)
    return maps


def _assemble(meta, results):
    N, PER = meta["N"], meta["PER"]
    out = np.zeros((N, D), np.float32)
    residual = np.zeros((N, D), np.float32)
    for c in range(NCORES):
        out[c * PER:(c + 1) * PER] = results[c]["out_sh"][:, :PER].T
        residual[c * PER:(c + 1) * PER] = results[c]["res_sh"][:, :PER].T
    return out, residual


def kernel(x, x_org, adj_values, Wi, bi, conv_W, conv_b, ln_g, ln_b, Wl, bl, Wres,
           edge_index):
    inputs = dict(x=x, x_org=x_org, adj_values=adj_values, Wi=Wi, bi=bi,
                  conv_W=conv_W, conv_b=conv_b, ln_g=ln_g, ln_b=ln_b, Wl=Wl,
                  bl=bl, Wres=Wres)
    meta = _preprocess(x, x_org, adj_values, edge_index)
    nc = _build_bass(meta)

    from concourse.bass_utils import run_bass_kernel_spmd

    trace = os.environ.get("GCN_TRACE", "0") == "1"
    res = run_bass_kernel_spmd(
        nc, _in_maps(meta, inputs), core_ids=list(range(NCORES)), trace=trace,
        trace_kwargs={"title": "gcn_encoder"} if trace else {},
    )
    if trace and res.exec_time_ns is not None:
        print(f"HW exec time: {res.exec_time_ns} ns")
        if res.instructions_and_trace:
            print(f"trace: {res.instructions_and_trace[1]}")
    return _assemble(meta, res.results)
